# revision 20
# baseline (speedup 1.0000x reference)
"""GraphWave (WaveNet-style dilated convs + ChebConv GNN) on 8 trn2 NeuronCores.

Whole network in ONE Bass SPMD kernel, node-parallel over the 8 cores:
  - nodes padded 20000 -> 20480 = 8 cores x 2560; per-core conv layout is
    [128 = 4 groups x 32 channels (partitions), T * 640 (free, t-major)]
  - activations/weights in bf16 (PSUM accumulation fp32), BN statistics fp32
  - dilated convs: full-width K=128 block-diagonal matmuls; tanh via
    2*sigmoid(2x)-1 (single ACT table); the 1/2 scale folds into BN scale
    invariance (with exact eps compensation) and a 2x on skip weights
  - BatchNorm: local fp32 stats + [32,2] AllReduce per layer
  - ChebConv: local features -> node-major bf16 DRAM slab -> AllGather full
    [20480, F] table -> dma_gather edge source rows -> segment-sum as
    one-hot matmuls accumulated in PSUM per 128-node destination block
    (edges sharded by destination, block-sorted on host)
Host does only: input instance-norm, edge preprocessing, weight packing,
final de-norm.
"""
import sys
import numpy as np

sys.path.insert(0, '/opt/trn_rl_repo')

EPS = 1e-5
DILATIONS = (1, 2, 1, 2, 1, 2, 1, 2)
GCN_AT = {1: 0, 5: 1}

N_NODES, T_IN, N_EDGES = 20000, 13, 200000
RC, SC, EC, HOR = 32, 256, 512, 12
NCORES = 8
NS = 2560             # padded nodes per core
NGRP = 4
NG = NS // NGRP       # 640
B_TILES = 12          # 128-edge tiles per 128-dest block (uniform, padded)
REAL_PER_CORE = N_NODES // NCORES


def _timeline():
    t = [T_IN]
    for d in DILATIONS:
        t.append(t[-1] - d)
    return t


T_SEQ = _timeline()


# ============================================================ device program
def build_nc(ns=NS, n_real_total=N_NODES, b_tiles=B_TILES, ncores=NCORES,
             real_per_core=REAL_PER_CORE, dbg=()):
    import concourse.bass as bass
    import concourse.tile as tile
    from concourse import bacc, mybir
    f32 = mybir.dt.float32
    bf16 = mybir.dt.bfloat16
    i16 = mybir.dt.int16
    AF = mybir.ActivationFunctionType
    OP = mybir.AluOpType
    AX = mybir.AxisListType

    ng = ns // NGRP
    nblk = ns // 128
    e_tiles = nblk * b_tiles
    n_idx = e_tiles * 128
    npad = ns * ncores
    cnk = 320 if ng % 320 == 0 else ng     # conv/dense chunk (within-group)
    bt2 = b_tiles // 2                     # gather granularity (half block)
    assert ng % cnk == 0 and b_tiles % 2 == 0

    def chunks(total, sz):
        out, o = [], 0
        while o < total:
            c = min(sz, total - o)
            out.append((o, c))
            o += c
        return out

    nc = bacc.Bacc("TRN2", target_bir_lowering=False, debug=False,
                   num_devices=ncores, enable_asserts=False,
                   num_swdge_queues=2)

    # ---------------- inputs ----------------
    xc_in = nc.dram_tensor("xc", [NGRP, T_IN * ng], bf16, kind="ExternalInput")
    gidx_in = nc.dram_tensor("gidx", [128, n_idx // 16], i16, kind="ExternalInput")
    colf_in = nc.dram_tensor("colf", [128, e_tiles], f32, kind="ExternalInput")
    nrmf_in = nc.dram_tensor("nrmf", [128, e_tiles], f32, kind="ExternalInput")
    cw_in = nc.dram_tensor("convw", [128, 8 * 2 * 2 * 128], bf16, kind="ExternalInput")
    cb_in = nc.dram_tensor("convb", [128, 16], f32, kind="ExternalInput")
    sw_in = nc.dram_tensor("skipw", [RC, 8 * SC], bf16, kind="ExternalInput")
    stw_in = nc.dram_tensor("startsel", [NGRP, 128], bf16, kind="ExternalInput")
    stb_in = nc.dram_tensor("startb128", [128, 1], f32, kind="ExternalInput")
    sbsum_in = nc.dram_tensor("skipbsum", [128, 2], f32, kind="ExternalInput")
    g0w_in = nc.dram_tensor("g0w", [128, 2 * 3 * 384], bf16, kind="ExternalInput")
    g0b_in = nc.dram_tensor("g0b", [128, 3], f32, kind="ExternalInput")
    g1w_in = nc.dram_tensor("g1w", [128, 2 * 2 * 192], bf16, kind="ExternalInput")
    g1b_in = nc.dram_tensor("g1b", [128, 2], f32, kind="ExternalInput")
    e1w_in = nc.dram_tensor("e1w", [128, 2 * EC], bf16, kind="ExternalInput")
    e1b_in = nc.dram_tensor("e1b", [128, 4], f32, kind="ExternalInput")
    e2w_in = nc.dram_tensor("e2w", [128, 4 * HOR], bf16, kind="ExternalInput")
    e2b_in = nc.dram_tensor("e2b", [HOR, 1], f32, kind="ExternalInput")
    iota_in = nc.dram_tensor("iotac", [128, 128], f32, kind="ExternalInput")
    ident_in = nc.dram_tensor("ident", [128, 128], f32, kind="ExternalInput")
    identb_in = nc.dram_tensor("identb", [128, 128], bf16, kind="ExternalInput")
    sel_in = nc.dram_tensor("sel", [128, RC], f32, kind="ExternalInput")
    sel2_in = nc.dram_tensor("sel2", [RC, 128], f32, kind="ExternalInput")

    out_d = nc.dram_tensor("out", [HOR, ns], f32, kind="ExternalOutput")
    dbg_d = {name: nc.dram_tensor(name, [128, t * ng], bf16,
                                  kind="ExternalOutput")
             for (name, t) in dbg}

    rg = [list(range(ncores))]

    with tile.TileContext(nc) as tc:
        import contextlib
        ctx = contextlib.ExitStack()
        wpool = ctx.enter_context(tc.tile_pool(name="wpool", bufs=1))
        hpool = ctx.enter_context(tc.tile_pool(name="hpool", bufs=2))
        spool = ctx.enter_context(tc.tile_pool(name="spool", bufs=1))
        vpool = ctx.enter_context(tc.tile_pool(name="vpool", bufs=2))
        tiny = ctx.enter_context(tc.tile_pool(name="tiny", bufs=2))
        ppa = ctx.enter_context(tc.tile_pool(name="ppa", bufs=2, space="PSUM"))
        ppc = ctx.enter_context(tc.tile_pool(name="ppc", bufs=2, space="PSUM"))
        ppd = ctx.enter_context(tc.tile_pool(name="ppd", bufs=2, space="PSUM"))
        ppt = ctx.enter_context(tc.tile_pool(name="ppt", bufs=2, space="PSUM"))
        dpool = ctx.enter_context(tc.tile_pool(name="dpool", bufs=1, space="DRAM"))

        # ------------ constants / weights -> SBUF ------------
        def loadp(name, src, shape, dtype=f32):
            t = wpool.tile(shape, dtype, name=name)
            nc.sync.dma_start(t[:], src[:])
            return t

        conv_w = loadp("conv_w", cw_in, [128, 8 * 2 * 2 * 128], bf16)
        conv_b = loadp("conv_b", cb_in, [128, 16])
        skip_w = loadp("skip_w", sw_in, [RC, 8 * SC], bf16)
        start_sel = loadp("start_sel", stw_in, [NGRP, 128], bf16)
        start_b = loadp("start_b", stb_in, [128, 1])
        sbsum = loadp("sbsum", sbsum_in, [128, 2])
        g0w = loadp("g0w", g0w_in, [128, 2 * 3 * 384], bf16)
        g0b = loadp("g0b", g0b_in, [128, 3])
        g1w = loadp("g1w", g1w_in, [128, 2 * 2 * 192], bf16)
        g1b = loadp("g1b", g1b_in, [128, 2])
        e1w = loadp("e1w", e1w_in, [128, 2 * EC], bf16)
        e1b = loadp("e1b", e1b_in, [128, 4])
        e2w = loadp("e2w", e2w_in, [128, 4 * HOR], bf16)
        e2b = loadp("e2b", e2b_in, [HOR, 1])
        iota = loadp("iota", iota_in, [128, 128])
        ident = loadp("ident", ident_in, [128, 128])
        identb = loadp("identb", identb_in, [128, 128], bf16)
        sel = loadp("sel", sel_in, [128, RC])
        sel2 = loadp("sel2", sel2_in, [RC, 128])
        gidx = loadp("gidx", gidx_in, [128, n_idx // 16], i16)
        colf = loadp("colf", colf_in, [128, e_tiles])
        nrmf = loadp("nrmf", nrmf_in, [128, e_tiles])
        xc_sb = hpool.tile([NGRP, T_IN * ng], bf16, name="xc_sb", tag="h",
                           padded_shape=[128, T_IN * ng])
        nc.sync.dma_start(xc_sb[:], xc_in[:])

        def dump(name, t_tile):
            if name in dbg_d:
                dt_ = dbg_d[name].ap().dtype
                if t_tile.dtype != dt_:
                    tmp = vpool.tile([128, t_tile.shape[1]], dt_,
                                     name=f"dmp_{name}", tag="dmp")
                    nc.vector.tensor_copy(tmp[:], t_tile[:])
                    nc.sync.dma_start(dbg_d[name][:, 0:t_tile.shape[1]], tmp[:])
                else:
                    nc.sync.dma_start(dbg_d[name][:, 0:t_tile.shape[1]], t_tile[:])

        # ------------ start conv: K=4 blockdiag matmul per chunk ------------
        # h0 is stored WITHOUT the start bias (BN is shift-invariant per
        # channel; the bias effect on layer-0 convs is folded into their
        # biases host-side).  Keeps h0 zero-mean so bf16 storage is cheap.
        h = hpool.tile([128, T_IN * ng], bf16, name="h0", tag="h")
        for (o, cz) in chunks(T_IN * ng, cnk):
            ps = ppc.tile([128, cnk], f32, name="ps0", tag="conv")
            nc.tensor.matmul(ps[:, 0:cz], start_sel[:], xc_sb[:, o:o + cz],
                             start=True, stop=True)
            nc.vector.tensor_copy(h[:, o:o + cz], ps[:, 0:cz])
        dump("h0", h)

        # ------------ BN (stats of X/2 in fp32, exact eps compensation) -----
        def bn_layer(h_t, t_len, li):
            pad_lo = real_per_core - 3 * ng
            if pad_lo < ng:
                nc.vector.memset(
                    h_t[:].rearrange("p (t n) -> p t n", t=t_len)[96:128, :, pad_lo:ng],
                    0.0)
            st = tiny.tile([128, 2], f32, name=f"st{li}", tag="st")
            nc.vector.tensor_reduce(st[:, 0:1], h_t[:], AX.X, OP.add)
            sqa = tiny.tile([128, t_len], f32, name=f"sqa{li}", tag="sqa")
            sqs = tiny.tile([128, ng], f32, name=f"sqs{li}", tag="sqs", bufs=1)
            for t in range(t_len):
                nc.scalar.activation(sqs[:], h_t[:, t * ng:(t + 1) * ng],
                                     AF.Square, accum_out=sqa[:, t:t + 1])
            nc.vector.tensor_reduce(st[:, 1:2], sqa[:, 0:t_len], AX.X, OP.add)
            ps = ppt.tile([RC, 2], f32, name=f"bnps{li}", tag="tr")
            nc.tensor.matmul(ps[:], sel[:], st[:], start=True, stop=True)
            st32 = tiny.tile([RC, 2], f32, name=f"st32_{li}", tag="st32")
            nc.vector.tensor_copy(st32[:], ps[:])
            bin_ = dpool.tile([RC, 2], f32, name=f"bnin{li}", tag=f"bnin{li}")
            bout = dpool.tile([RC, 2], f32, name=f"bnout{li}", tag=f"bnout{li}")
            nc.sync.dma_start(bin_[:], st32[:])
            nc.gpsimd.collective_compute(
                "AllReduce", OP.add, replica_groups=rg,
                ins=[bin_[:].opt()], outs=[bout[:].opt()])
            stg = tiny.tile([RC, 2], f32, name=f"stg{li}", tag="st32")
            nc.sync.dma_start(stg[:], bout[:])
            cnt = float(n_real_total * t_len)
            mv = tiny.tile([RC, 2], f32, name=f"mv{li}", tag="st32")
            nc.vector.tensor_scalar(mv[:], stg[:], 1.0 / cnt, None, op0=OP.mult)
            # stats are of X/2; reference normalizes X with eps inside sqrt:
            # (x' - m') * 2 / sqrt(4*var' + EPS)  ==  (X - m)/sqrt(var + EPS)
            m2 = tiny.tile([RC, 1], f32, name=f"m2_{li}", tag="var")
            nc.vector.tensor_tensor(m2[:], mv[:, 0:1], mv[:, 0:1], op=OP.mult)
            var = tiny.tile([RC, 1], f32, name=f"var{li}", tag="var")
            nc.vector.tensor_tensor(var[:], mv[:, 1:2], m2[:], op=OP.subtract)
            var4 = tiny.tile([RC, 1], f32, name=f"var4{li}", tag="var")
            nc.vector.tensor_scalar(var4[:], var[:], 4.0, float(EPS),
                                    op0=OP.mult, op1=OP.add)
            sd = tiny.tile([RC, 1], f32, name=f"sd{li}", tag="var")
            nc.scalar.activation(sd[:], var4[:], AF.Sqrt)
            isd = tiny.tile([RC, 1], f32, name=f"isd{li}", tag="var")
            nc.vector.reciprocal(isd[:], sd[:])
            sc2 = tiny.tile([RC, 2], f32, name=f"sc2_{li}", tag="st32")
            nc.vector.tensor_copy(sc2[:, 0:1], mv[:, 0:1])
            nc.vector.tensor_scalar(sc2[:, 1:2], isd[:], 2.0, None, op0=OP.mult)
            ps2 = ppt.tile([128, 2], f32, name=f"bps{li}", tag="tr")
            nc.tensor.matmul(ps2[:], sel2[:], sc2[:], start=True, stop=True)
            sc128 = tiny.tile([128, 2], f32, name=f"sc128_{li}", tag="st")
            nc.vector.tensor_copy(sc128[:], ps2[:])
            out = hpool.tile([128, t_len * ng], bf16, name=f"hbn{li}", tag="h")
            nc.vector.tensor_scalar(out[:], h_t[:], sc128[:, 0:1], sc128[:, 1:2],
                                    op0=OP.subtract, op1=OP.mult)
            return out

        # ------------ ChebConv ------------
        def cheb(h_t, t_len, li, wT, bT, fchunks, fpad):
            F = RC * t_len
            nk = len(fchunks)
            xfT = [spool.tile([128, ns], bf16, name=f"xfT{li}_{k}", tag=f"xfT{k}")
                   for k in range(nk)]
            for t in range(t_len):
                k, r = (t * RC) // 128, (t * RC) % 128
                for g in range(NGRP):
                    nc.vector.tensor_copy(
                        xfT[k][r:r + RC, g * ng:(g + 1) * ng],
                        h_t[32 * g:32 * g + 32, t * ng:(t + 1) * ng])
            slab = dpool.tile([ns, fpad], bf16, name=f"slab{li}", tag=f"slab{li}")
            for nb in range(nblk):
                nm = vpool.tile([128, fpad], bf16, name=f"nm{li}", tag="nm")
                if fpad > F:
                    nc.vector.memset(nm[:, F:fpad], 0.0)
                for k, (r0, rr) in enumerate(fchunks):
                    pst = ppt.tile([128, 128], bf16, name=f"pst{li}", tag="tr")
                    nc.tensor.matmul(pst[0:128, 0:rr],
                                     xfT[k][0:rr, nb * 128:(nb + 1) * 128],
                                     identb[0:rr, 0:rr], is_transpose=True)
                    nc.vector.tensor_copy(nm[:, r0:r0 + rr], pst[0:128, 0:rr])
                nc.sync.dma_start(slab[nb * 128:(nb + 1) * 128, :], nm[:])
            full = dpool.tile([npad, fpad], bf16, name=f"full{li}",
                              tag=f"full{li}",
                              addr_space="Shared" if ncores > 4 else "Local")
            nc.gpsimd.collective_compute(
                "AllGather", OP.bypass, replica_groups=rg,
                ins=[slab[:].opt()], outs=[full[:].opt()])
            txT = [spool.tile([128, ns], bf16, name=f"txT{li}_{k}", tag=f"txT{k}")
                   for k in range(nk)]
            for nb in range(nblk):
                acc = ppa.tile([128, fpad], f32, name=f"acc{li}", tag="acc")
                for hh in range(2):
                    V = vpool.tile([128, bt2, fpad], bf16, name=f"V{li}", tag="V")
                    i0 = nb * b_tiles + hh * bt2
                    nc.gpsimd.dma_gather(
                        V[:], full[:], gidx[:, i0 * 8:(i0 + bt2) * 8],
                        bt2 * 128, bt2 * 128, fpad, queue_num=hh)
                    for j in range(bt2):
                        et = i0 + j
                        M = vpool.tile([128, 128], bf16, name=f"M{li}", tag="M")
                        nc.vector.tensor_scalar(
                            M[:], iota[:], colf[:, et:et + 1], nrmf[:, et:et + 1],
                            op0=OP.is_equal, op1=OP.mult)
                        nc.tensor.matmul(acc[:], M[:], V[:, j, :],
                                         start=(hh == 0 and j == 0),
                                         stop=(hh == 1 and j == bt2 - 1))
                tnm = vpool.tile([128, F], f32, name=f"tnm{li}", tag="nm")
                nc.vector.tensor_copy(tnm[:], acc[:, 0:F])
                for k, (r0, rr) in enumerate(fchunks):
                    pst = ppt.tile([128, 128], f32, name=f"pst2{li}", tag="tr")
                    nc.tensor.matmul(pst[0:rr, 0:128], tnm[:, r0:r0 + rr],
                                     ident[:, :], is_transpose=True)
                    nc.vector.tensor_copy(txT[k][0:rr, nb * 128:(nb + 1) * 128],
                                          pst[0:rr, 0:128])  # cast f32->bf16
            # dense: out = W0p^T xfT + W1p'^T txT + b, written in conv layout
            out = hpool.tile([128, t_len * ng], bf16, name=f"hch{li}", tag="h")
            wv = wT[:].rearrange("p (w k o) -> w k p o", w=2, k=nk)
            for ko, (o0, oo) in enumerate(fchunks):
                for g in range(NGRP):
                    for (no, cz) in chunks(ng, cnk):
                        nn0 = g * ng + no
                        psd = ppd.tile([128, cnk], f32, name=f"psd{li}", tag="dense")
                        for ki, (r0, rr) in enumerate(fchunks):
                            nc.tensor.matmul(
                                psd[0:oo, 0:cz],
                                wv[0, ki, 0:rr, o0:o0 + oo],
                                xfT[ki][0:rr, nn0:nn0 + cz],
                                start=(ki == 0), stop=False)
                            nc.tensor.matmul(
                                psd[0:oo, 0:cz],
                                wv[1, ki, 0:rr, o0:o0 + oo],
                                txT[ki][0:rr, nn0:nn0 + cz],
                                start=False, stop=(ki == nk - 1))
                        for band in range(oo // 32):
                            fo = o0 + band * 32
                            t_o = fo // RC
                            nc.vector.tensor_scalar(
                                out[32 * g:32 * g + 32,
                                    t_o * ng + no:t_o * ng + no + cz],
                                psd[band * 32:(band + 1) * 32, 0:cz],
                                bT[:, ko:ko + 1][band * 32:(band + 1) * 32],
                                None, op0=OP.add)
            return out

        # ------------ layers ------------
        skip_acc = spool.tile([128, 2 * ns], f32, name="skip_acc", tag="skip")
        for li, d in enumerate(DILATIONS):
            t_in = T_SEQ[li]
            t_out = t_in - d
            if li in GCN_AT:
                if GCN_AT[li] == 0:
                    h = cheb(h, t_in, li, g0w, g0b,
                             [(0, 128), (128, 128), (256, 128)], 384)
                else:
                    h = cheb(h, t_in, li, g1w, g1b, [(0, 128), (128, 64)], 256)
                dump(f"ch{li}", h)
            cwv = conv_w[:].rearrange("p (l t f o) -> l t f p o", l=8, t=2, f=2)
            fb = conv_b[:, 2 * li:2 * li + 1]        # [128,1] (2x filter bias)
            gb = conv_b[:, 2 * li + 1:2 * li + 2]    # [128,1]
            swv = skip_w[:].rearrange("c (l o) -> l c o", l=8, o=SC)
            hn = hpool.tile([128, t_out * ng], f32, name=f"hn{li}", tag="hn",
                            bufs=1)
            hl = tiny.tile([RC, ns], bf16, name=f"hl{li}", tag="hl", bufs=1)
            for (o, cz) in chunks(t_out * ng, cnk):
                psf = ppc.tile([128, cnk], f32, name=f"cpf{li}", tag="conv")
                psg = ppc.tile([128, cnk], f32, name=f"cpg{li}", tag="conv")
                for fg, pst_ in ((0, psf), (1, psg)):
                    nc.tensor.matmul(
                        pst_[:, 0:cz], cwv[li, 0, fg],
                        h[:, o:o + cz], start=True, stop=False)
                    nc.tensor.matmul(
                        pst_[:, 0:cz], cwv[li, 1, fg],
                        h[:, d * ng + o:d * ng + o + cz], start=False, stop=True)
                fF = tiny.tile([128, cnk], f32, name=f"fF{li}", tag="cf", bufs=3)
                nc.scalar.activation(fF[:, 0:cz], psf[:, 0:cz], AF.Sigmoid,
                                     bias=fb, scale=2.0)
                hs = hn[:, o:o + cz]
                nc.scalar.activation(hs, psg[:, 0:cz], AF.Sigmoid, bias=gb)
                # hs = (fF - 0.5) * hs   == (f*g)/2
                nc.vector.scalar_tensor_tensor(hs, fF[:, 0:cz], 0.5, hs,
                                               op0=OP.subtract, op1=OP.mult)
                if o >= (t_out - 1) * ng:   # last time col: stash for skip conv
                    no = o - (t_out - 1) * ng
                    for g in range(NGRP):
                        nc.vector.tensor_copy(
                            hl[:, g * ng + no:g * ng + no + cz],
                            hn[32 * g:32 * g + 32, o:o + cz])
                # hs += 0.5 * residual
                nc.vector.scalar_tensor_tensor(
                    hs, h[:, d * ng + o:d * ng + o + cz],
                    0.5, hs, op0=OP.mult, op1=OP.add)
            for oc in range(2):
                for (no, cz) in chunks(ns, cnk):
                    ps2 = ppd.tile([128, cnk], f32, name=f"sps{li}", tag="dense")
                    nc.tensor.matmul(
                        ps2[:, 0:cz],
                        swv[li, :, oc * 128:(oc + 1) * 128],
                        hl[:, no:no + cz], start=True, stop=True)
                    dst = skip_acc[:, oc * ns + no:oc * ns + no + cz]
                    if li == 0:
                        nc.vector.tensor_copy(dst, ps2[:, 0:cz])
                    else:
                        nc.vector.tensor_tensor(dst, dst, ps2[:, 0:cz], op=OP.add)
            dump(f"hn{li}", hn)
            h = bn_layer(hn, t_out, li)
            dump(f"bn{li}", h)

        # ------------ relu(skip)+bias (bf16), end MLP ------------
        relu_b = spool.tile([128, 2 * ns], bf16, name="relu_b", tag="skipb")
        for oc in range(2):
            nc.vector.tensor_scalar(
                relu_b[:, oc * ns:(oc + 1) * ns],
                skip_acc[:, oc * ns:(oc + 1) * ns],
                sbsum[:, oc:oc + 1], 0.0, op0=OP.add, op1=OP.max)
        e1v = e1w[:].rearrange("p (k o) -> k p o", k=2)
        e2v = e2w[:].rearrange("p (k o) -> k p o", k=4)
        for (no, cz) in chunks(ns, cnk):
            e1c = vpool.tile([128, 4, cnk], bf16, name="e1c", tag="V")
            for m in range(4):
                ps = ppd.tile([128, cnk], f32, name="e1ps", tag="dense")
                for k in range(2):
                    nc.tensor.matmul(
                        ps[:, 0:cz], e1v[k, :, m * 128:(m + 1) * 128],
                        relu_b[:, k * ns + no:k * ns + no + cz],
                        start=(k == 0), stop=(k == 1))
                nc.vector.tensor_scalar(e1c[:, m, 0:cz], ps[:, 0:cz],
                                        e1b[:, m:m + 1], None, op0=OP.add)
            ps3 = ppc.tile([HOR, cnk], f32, name="e2ps", tag="conv")
            for k in range(4):
                nc.tensor.matmul(ps3[:, 0:cz], e2v[k], e1c[:, k, 0:cz],
                                 start=(k == 0), stop=(k == 3))
            ob = vpool.tile([HOR, cnk], f32, name="ob", tag="ob")
            nc.vector.tensor_scalar(ob[:, 0:cz], ps3[:, 0:cz], e2b[:], None,
                                    op0=OP.add)
            nc.sync.dma_start(out_d[:, no:no + cz], ob[:, 0:cz])

        ctx.close()

    nc.compile()
    return nc


# ============================================================ host side
_NC_CACHE = {}


def get_nc(key="full", **kw):
    if key not in _NC_CACHE:
        _NC_CACHE[key] = build_nc(**kw)
    return _NC_CACHE[key]


def host_prep(x, edge_index, edge_attr, weights, ns=NS, b_tiles=B_TILES,
              ncores=NCORES, real_per_core=REAL_PER_CORE):
    import ml_dtypes
    bf = ml_dtypes.bfloat16
    ng = ns // NGRP
    nblk = ns // 128
    e_tiles = nblk * b_tiles
    n_real = ncores * real_per_core
    x = np.asarray(x, np.float32).reshape(n_real, T_IN)
    means = x.mean(axis=1, keepdims=True)
    xc = x - means
    stdev = np.sqrt(xc.var(axis=1) + EPS)[:, None]
    xc = xc / stdev

    row = np.asarray(edge_index[0]).astype(np.int64)
    col = np.asarray(edge_index[1]).astype(np.int64)
    w = np.where(row == col, 0.0, np.asarray(edge_attr, np.float32)).astype(np.float32)
    deg = np.bincount(row, weights=w, minlength=n_real).astype(np.float32)
    dinv = np.where(deg > 0, 1.0 / np.sqrt(np.where(deg > 0, deg, 1.0)), 0.0
                    ).astype(np.float32)
    norm = (dinv[row] * w * dinv[col]).astype(np.float32)

    src_pad = (row + (ns - real_per_core) * (row // real_per_core)).astype(np.int64)
    dst_core = col // real_per_core
    dst_loc = col - dst_core * real_per_core
    dst_blk = dst_loc // 128
    dst_off = dst_loc % 128

    per_core = []
    for c in range(ncores):
        m = dst_core == c
        sp, db, do, nm = src_pad[m], dst_blk[m], dst_off[m], norm[m]
        order = np.argsort(db, kind='stable')
        sp, db, do, nm = sp[order], db[order], do[order], nm[order]
        cnt = np.bincount(db, minlength=nblk)
        if cnt.max(initial=0) > b_tiles * 128:
            raise RuntimeError(f"B_TILES too small: {cnt.max()} > {b_tiles * 128}")
        idx = np.zeros(e_tiles * 128, np.int16)
        cof = np.full(e_tiles * 128, -1.0, np.float32)
        nrm = np.zeros(e_tiles * 128, np.float32)
        st = 0
        for b in range(nblk):
            k = int(cnt[b])
            base = b * b_tiles * 128
            idx[base:base + k] = sp[st:st + k]
            cof[base:base + k] = do[st:st + k]
            nrm[base:base + k] = nm[st:st + k]
            st += k
        iw = np.tile(idx.reshape(-1, 16).T, (8, 1))
        per_core.append(dict(idx=np.ascontiguousarray(iw),
                             colf=np.ascontiguousarray(cof.reshape(-1, 128).T),
                             nrmf=np.ascontiguousarray(nrm.reshape(-1, 128).T)))

    xcp = np.zeros((ncores * ns, T_IN), np.float32)
    for c in range(ncores):
        xcp[c * ns:c * ns + real_per_core] = \
            xc[c * real_per_core:(c + 1) * real_per_core]
    xc_cores = []
    for c in range(ncores):
        blockc = xcp[c * ns:(c + 1) * ns]
        g = blockc.reshape(NGRP, ng, T_IN).transpose(0, 2, 1)  # [4, 13, ng]
        xc_cores.append(np.ascontiguousarray(g.reshape(NGRP, -1)).astype(bf))

    wts = {}
    fW = np.asarray(weights['filter_W'], np.float32)
    fb = np.asarray(weights['filter_b'], np.float32)
    gW = np.asarray(weights['gate_W'], np.float32)
    gb = np.asarray(weights['gate_b'], np.float32)
    stb = np.asarray(weights['start_b'], np.float32).reshape(RC)
    corr_f0 = (fW[0, :, :, 0] + fW[0, :, :, 1]) @ stb    # [32]
    corr_g0 = (gW[0, :, :, 0] + gW[0, :, :, 1]) @ stb
    cw = np.zeros((8, 2, 2, 128, 128), np.float32)   # [li, tap, fg, (g,c), (g,o)]
    cb = np.zeros((128, 16), np.float32)
    for li in range(8):
        for tap in range(2):
            for g in range(NGRP):
                s = slice(32 * g, 32 * g + 32)
                cw[li, tap, 0, s, s] = fW[li, :, :, tap].T
                cw[li, tap, 1, s, s] = gW[li, :, :, tap].T
        fbl = fb[li] + (corr_f0 if li == 0 else 0.0)
        gbl = gb[li] + (corr_g0 if li == 0 else 0.0)
        cb[:, 2 * li] = np.tile(2.0 * fbl, NGRP)
        cb[:, 2 * li + 1] = np.tile(gbl, NGRP)
    wts['convw'] = np.ascontiguousarray(
        cw.transpose(3, 0, 1, 2, 4).reshape(128, -1)).astype(bf)
    wts['convb'] = cb
    sW = np.asarray(weights['skip_W'], np.float32)
    sb = np.asarray(weights['skip_b'], np.float32)
    wts['skipw'] = np.ascontiguousarray(
        (2.0 * sW.transpose(0, 2, 1)).transpose(1, 0, 2).reshape(RC, -1)
        ).astype(bf)
    wts['skipbsum'] = np.ascontiguousarray(sb.sum(axis=0).reshape(2, 128).T)
    stW = np.asarray(weights['start_W'], np.float32).reshape(RC)
    ssel = np.zeros((NGRP, 128), np.float32)
    for g in range(NGRP):
        ssel[g, 32 * g:32 * g + 32] = stW
    wts['startsel'] = ssel.astype(bf)
    wts['startb128'] = np.ascontiguousarray(
        np.tile(np.asarray(weights['start_b'], np.float32).reshape(RC), NGRP
                ).reshape(128, 1))

    def gperm(W0, W1, b, t_len):
        F = RC * t_len
        pi = np.empty(F, np.int64)
        for t in range(t_len):
            for ch in range(RC):
                pi[t * RC + ch] = ch * t_len + t
        W0p = W0[np.ix_(pi, pi)].astype(np.float32)
        W1p = (-W1[np.ix_(pi, pi)]).astype(np.float32)
        bp = b[pi].astype(np.float32)
        return W0p, W1p, bp

    W0p, W1p, g0bp = gperm(np.asarray(weights['gcn0_W0'], np.float64),
                           np.asarray(weights['gcn0_W1'], np.float64),
                           np.asarray(weights['gcn0_b'], np.float64), 12)
    g0pack = np.stack([W0p.reshape(3, 128, 384), W1p.reshape(3, 128, 384)])
    wts['g0w'] = np.ascontiguousarray(
        g0pack.transpose(2, 0, 1, 3).reshape(128, -1)).astype(bf)
    wts['g0b'] = np.ascontiguousarray(g0bp.reshape(3, 128).T)
    W0p, W1p, g1bp = gperm(np.asarray(weights['gcn1_W0'], np.float64),
                           np.asarray(weights['gcn1_W1'], np.float64),
                           np.asarray(weights['gcn1_b'], np.float64), 6)
    g1pack = np.zeros((2, 2, 128, 192), np.float32)
    for wi, Wp in enumerate([W0p, W1p]):
        g1pack[wi, 0, :, :] = Wp[0:128]
        g1pack[wi, 1, 0:64, :] = Wp[128:192]
    wts['g1w'] = np.ascontiguousarray(
        g1pack.transpose(2, 0, 1, 3).reshape(128, -1)).astype(bf)
    g1bpad = np.zeros((2, 128), np.float32)
    g1bpad[0] = g1bp[0:128]
    g1bpad[1, 0:64] = g1bp[128:192]
    wts['g1b'] = np.ascontiguousarray(g1bpad.T)
    e1W = np.asarray(weights['end1_W'], np.float32)
    wts['e1w'] = np.ascontiguousarray(
        e1W.T.reshape(2, 128, EC).transpose(1, 0, 2).reshape(128, -1)).astype(bf)
    wts['e1b'] = np.ascontiguousarray(
        np.asarray(weights['end1_b'], np.float32).reshape(4, 128).T)
    e2W = np.asarray(weights['end2_W'], np.float32)
    wts['e2w'] = np.ascontiguousarray(
        e2W.T.reshape(4, 128, HOR).transpose(1, 0, 2).reshape(128, -1)).astype(bf)
    wts['e2b'] = np.ascontiguousarray(
        np.asarray(weights['end2_b'], np.float32).reshape(HOR, 1))
    wts['iotac'] = np.tile(np.arange(128, dtype=np.float32)[None, :], (128, 1))
    wts['ident'] = np.eye(128, dtype=np.float32)
    wts['identb'] = np.eye(128, dtype=np.float32).astype(bf)
    selm = np.zeros((128, RC), np.float32)
    selm[np.arange(128), np.arange(128) % RC] = 1.0
    wts['sel'] = selm
    wts['sel2'] = np.ascontiguousarray(selm.T)

    in_maps = []
    for c in range(ncores):
        im = dict(wts)
        im['xc'] = xc_cores[c]
        im['gidx'] = per_core[c]['idx']
        im['colf'] = per_core[c]['colf']
        im['nrmf'] = per_core[c]['nrmf']
        in_maps.append(im)
    return in_maps, means, stdev


def kernel(x, edge_index, edge_attr, start_W, start_b, filter_W, filter_b,
           gate_W, gate_b, skip_W, skip_b, gcn0_W0, gcn0_W1, gcn0_b,
           gcn1_W0, gcn1_W1, gcn1_b, end1_W, end1_b, end2_W, end2_b):
    from concourse import bass_utils
    weights = dict(start_W=start_W, start_b=start_b, filter_W=filter_W,
                   filter_b=filter_b, gate_W=gate_W, gate_b=gate_b,
                   skip_W=skip_W, skip_b=skip_b, gcn0_W0=gcn0_W0,
                   gcn0_W1=gcn0_W1, gcn0_b=gcn0_b, gcn1_W0=gcn1_W0,
                   gcn1_W1=gcn1_W1, gcn1_b=gcn1_b, end1_W=end1_W,
                   end1_b=end1_b, end2_W=end2_W, end2_b=end2_b)
    in_maps, means, stdev = host_prep(x, edge_index, edge_attr, weights)
    nc = get_nc("full")
    res = bass_utils.run_bass_kernel_spmd(nc, in_maps,
                                          core_ids=list(range(NCORES)))
    out = np.empty((N_NODES, HOR), np.float32)
    for c in range(NCORES):
        o = res.results[c]["out"]
        out[c * REAL_PER_CORE:(c + 1) * REAL_PER_CORE] = o[:, :REAL_PER_CORE].T
    out = out[:, :, None] * stdev[:, :, None] + means[:, :, None]
    return np.ascontiguousarray(out.astype(np.float32))
